# revision 17
# baseline (speedup 1.0000x reference)
"""Trainium2 Bass kernel for NetGIN: 5-layer GIN (eps=0) + global mean pool +
per-layer linear heads + tanh, distributed over 8 NeuronCores.

Strategy (SPMD, identical program on all cores):
- Whole graphs are assigned to cores (snake deal by size desc). Per-core node
  columns are grouped by padded graph-size class; class counts are maxed over
  cores (ghost slots) so every core runs the same program shape.
- q = h @ w1 is computed BEFORE message passing (linearity of segment_sum), so
  only DIM=10 features move per edge instead of 128.
- Edges live on their src core. The q^T table is replicated into 8 row-bands
  of 16 partitions (one band per dst core). One batched ap_gather per chunk
  fetches q[src] for all 8 dst-core groups at once; a degree-class padded-CSR
  layout + batched tensor_reduce produces per-dst partial sums; a second
  ap_gather permutes class slots into the dst core's canonical column order;
  ReduceScatter(add) then hands each core its own rows summed over all cores.
- Combine: z = q + agg; u = relu(z + b1); h' = relu(w2^T u + b2) on PE/ACT.
- Pooling: per-size-class tensor_reduce -> graph slots -> ap_gather into
  global graph-id positions -> AllReduce over [16, 5*512] -> recip multiply ->
  5 accumulating PSUM matmuls with l_l -> tanh -> [16, 512] output per core.
"""
import math
import os
import sys

sys.path.insert(0, "/opt/trn_rl_repo")
from contextlib import ExitStack

import numpy as np

import concourse.bass as bass
import concourse.tile as tile
from concourse import mybir
from concourse._compat import with_exitstack
from concourse.bass_test_utils import run_kernel

NCORES = 8
DIM = 10
OUT = 16
NLAYERS = 5
G_TOTAL = 512
F_IN = 128

BASE_LADDER = [1, 2, 3, 4, 5, 6, 7, 8, 10, 12, 16, 20, 24, 32, 48, 64, 96, 128]
GCH = 2048  # edge-gather chunk (#indices)
PCH = 2048  # permute-gather chunk
MMCH = 512  # matmul column chunk (1 PSUM bank in fp32)

LAST_RESULTS = None


def _wrap_bands(flat):
    """[ngroups, M] -> [16*ngroups, M//16] int16; idx j of group g lands at
    row 16g + j%16, col j//16 (ap_gather index layout)."""
    ngroups, M = flat.shape
    assert M % 16 == 0
    out = np.zeros((16 * ngroups, M // 16), np.int16)
    for g in range(ngroups):
        for r in range(16):
            out[16 * g + r, :] = flat[g, r::16]
    return out


def build_plan(x, edge_index, batch, weights, G):
    N, F = x.shape
    assert F == F_IN
    assert G % 16 == 0

    counts = np.bincount(batch, minlength=G).astype(np.int64)
    gstart = np.zeros(G + 1, np.int64)
    gstart[1:] = np.cumsum(counts)

    # snake-deal graphs to cores by size desc
    order = np.argsort(-counts, kind="stable")
    assign = np.zeros(G, np.int64)
    for i, gid in enumerate(order):
        r, cpos = divmod(i, NCORES)
        assign[gid] = cpos if (r % 2 == 0) else (NCORES - 1 - cpos)

    # size classes (graph sizes padded to mult of 16), uniform across cores
    s_of_g = np.maximum(16, ((counts + 15) // 16) * 16)
    sizes = np.unique(s_of_g)[::-1].astype(np.int64)
    percore = {int(s): [[] for _ in range(NCORES)] for s in sizes}
    for gid in range(G):
        percore[int(s_of_g[gid])][int(assign[gid])].append(gid)
    size_classes = []  # (goff, m_s, s, coloff)
    slot_of_gid = np.zeros(G, np.int64)
    col0_of_gid = np.zeros(G, np.int64)
    goff = 0
    coloff = 0
    for s in sizes:
        s = int(s)
        m_s = max(len(percore[s][k]) for k in range(NCORES))
        size_classes.append((goff, m_s, s, coloff))
        for k in range(NCORES):
            for j, gid in enumerate(percore[s][k]):
                slot_of_gid[gid] = goff + j
                col0_of_gid[gid] = coloff + j * s
        goff += m_s
        coloff += m_s * s
    NGL = goff
    L = coloff
    assert L % 16 == 0 and L + 16 < 32768

    core_of_node = assign[batch]
    col_of_node = col0_of_gid[batch] + (np.arange(N) - gstart[batch])

    # ---- edge plan ----
    src, dst = edge_index[0].astype(np.int64), edge_index[1].astype(np.int64)
    e_owner = core_of_node[src]
    e_group = core_of_node[dst]
    e_scol = col_of_node[src]
    key = (e_owner * NCORES + e_group) * L + col_of_node[dst]
    eord = np.argsort(key, kind="stable")
    sk = key[eord]
    s_src = e_scol[eord]
    uk, ufirst, ucnt = np.unique(sk, return_index=True, return_counts=True)
    u_bucket = uk // L
    u_core = u_bucket // NCORES
    u_group = u_bucket % NCORES
    u_dcol = uk % L

    ladder = list(BASE_LADDER)
    maxdeg = int(ucnt.max()) if len(ucnt) else 1
    while ladder[-1] < maxdeg:
        ladder.append(ladder[-1] * 2)
    C = len(ladder)
    ladder_arr = np.array(ladder, np.int64)
    u_class = np.searchsorted(ladder_arr, ucnt)

    bucket_cls = u_bucket * C + u_class
    cnt3 = np.bincount(bucket_cls, minlength=NCORES * NCORES * C)
    n_raw_max = cnt3.reshape(NCORES * NCORES, C).max(axis=0)
    n_c = np.zeros(C, np.int64)
    for ci, c in enumerate(ladder):
        # 32-element alignment: GPSIMD idx APs (int16) must start 4B-aligned
        u16 = 32 // math.gcd(c, 32)
        n_c[ci] = ((int(n_raw_max[ci]) + u16 - 1) // u16) * u16
    class_off = np.zeros(C + 1, np.int64)
    pcls_off = np.zeros(C + 1, np.int64)
    for ci in range(C):
        class_off[ci + 1] = class_off[ci] + n_c[ci] * ladder[ci]
        pcls_off[ci + 1] = pcls_off[ci] + n_c[ci]
    W = int(class_off[C])
    NCLS = int(pcls_off[C])
    assert NCLS + 16 < 32768 and W % 16 == 0

    # slot of each unique dst within its (core, group, class) segment
    U = len(uk)
    ord2 = np.lexsort((u_dcol, u_class, u_group, u_core))
    sid2 = bucket_cls[ord2]
    newseg = np.ones(U, bool)
    newseg[1:] = sid2[1:] != sid2[:-1]
    seg_first = np.maximum.accumulate(np.where(newseg, np.arange(U), 0))
    rank2 = np.arange(U) - seg_first
    slot = np.empty(U, np.int64)
    slot[ord2] = rank2

    eu = np.repeat(np.arange(U), ucnt)
    within = np.arange(len(sk)) - np.repeat(ufirst, ucnt)
    tgt = class_off[u_class[eu]] + slot[eu] * ladder_arr[u_class[eu]] + within

    EIDXf = np.full((NCORES, NCORES, W), L, np.int64)  # sentinel col L (zeros)
    EIDXf[u_core[eu], u_group[eu], tgt] = s_src
    PIDXf = np.full((NCORES, NCORES, L), NCLS, np.int64)  # sentinel slot NCLS
    PIDXf[u_core, u_group, u_dcol] = pcls_off[u_class] + slot

    POOLf = np.full((NCORES, G), NGL, np.int64)
    POOLf[assign, np.arange(G)] = slot_of_gid

    # gather chunk schedule (same for every core/group)
    chunks = []  # (eoff, nd, c, poff)
    for ci, c in enumerate(ladder):
        if n_c[ci] == 0:
            continue
        u16 = 32 // math.gcd(c, 32)
        step = max(((GCH // c) // u16) * u16, u16)
        off = 0
        while off < n_c[ci]:
            nd = int(min(step, n_c[ci] - off))
            eoff = int(class_off[ci] + off * c)
            assert eoff % 32 == 0, (eoff, c, off)
            chunks.append((eoff, nd, c, int(pcls_off[ci] + off)))
            off += nd
    GW = max(nd * c for (_, nd, c, _) in chunks)

    # ---- per-core input tensors ----
    XT = np.zeros((NCORES, F, L), np.float32)
    XT[core_of_node, :, col_of_node] = x

    recip = (1.0 / np.maximum(counts, 1)).astype(np.float32)
    RECIP = np.tile(recip[None, :], (16, 1))

    W1R1 = np.zeros((128, 128), np.float32)
    WREP = np.zeros((128, 512), np.float32)
    for g in range(8):
        W1R1[:, 16 * g : 16 * g + DIM] = weights["w1_1"]
    for li in range(2, NLAYERS + 1):
        for g in range(8):
            WREP[64 : 64 + DIM, 128 * (li - 2) + 16 * g : 128 * (li - 2) + 16 * g + DIM] = weights[f"w1_{li}"]
    for li in range(1, NLAYERS + 1):
        WREP[32 : 32 + DIM, DIM * (li - 1) : DIM * li] = weights[f"w2_{li}"]
        WREP[32 : 32 + DIM, 50 + li - 1] = weights[f"b1_{li}"]
        WREP[0:DIM, OUT * (li - 1) : OUT * li] = weights[f"l_{li}"]
        WREP[0:DIM, 80 + li - 1] = weights[f"b2_{li}"]

    ins_list = []
    for k in range(NCORES):
        ins_list.append(
            dict(
                xt=XT[k],
                eidx=_wrap_bands(EIDXf[k]),
                pidx=_wrap_bands(PIDXf[k]),
                plidx=_wrap_bands(POOLf[k : k + 1]),
                recip=RECIP,
                w1r1=W1R1,
                wrep=WREP,
            )
        )

    return dict(
        L=L, W=W, NCLS=NCLS, NGL=NGL, G=G, GW=GW,
        chunks=chunks, size_classes=size_classes, ins_list=ins_list,
    )


def make_kern(plan, debug=False):
    L = plan["L"]
    W = plan["W"]
    NCLS = plan["NCLS"]
    NGL = plan["NGL"]
    G = plan["G"]
    GW = plan["GW"]
    chunks = plan["chunks"]
    size_classes = plan["size_classes"]
    f32 = mybir.dt.float32
    i16 = mybir.dt.int16

    @with_exitstack
    def kern(ctx: ExitStack, tc: tile.TileContext, outs, ins):
        nc = tc.nc
        if debug:
            (o_out, o_tab, o_part, o_rs, o_h) = outs
        else:
            (o_out,) = outs
        persist = ctx.enter_context(tc.tile_pool(name="persist", bufs=1))
        wpool = ctx.enter_context(tc.tile_pool(name="wp", bufs=2))
        spool = ctx.enter_context(tc.tile_pool(name="sp", bufs=2))
        pspool = ctx.enter_context(tc.tile_pool(name="ps", bufs=2, space="PSUM"))
        ps2pool = ctx.enter_context(tc.tile_pool(name="ps2", bufs=2, space="PSUM"))
        dram = ctx.enter_context(tc.tile_pool(name="dr", bufs=1, space="DRAM"))

        T_TAB = persist.tile([128, L + 16], f32)
        T_PART = persist.tile([128, NCLS + 16], f32)
        T_SCR = persist.tile([128, L], f32)
        T_EIDX = persist.tile([128, W // 16], i16)
        T_PIDX = persist.tile([128, L // 16], i16)
        T_PLIDX = persist.tile([16, G // 16], i16)
        T_HPOOL = persist.tile([16, NGL + 16], f32)
        T_RECIP = persist.tile([16, G], f32)
        T_W1R1 = persist.tile([128, 128], f32)
        T_WREP = persist.tile([128, 512], f32)

        nc.vector.memset(T_TAB[:], 0.0)
        nc.vector.memset(T_PART[:], 0.0)
        nc.vector.memset(T_HPOOL[:], 0.0)
        nc.sync.dma_start(out=T_EIDX[:], in_=ins["eidx"][:])
        nc.sync.dma_start(out=T_PIDX[:], in_=ins["pidx"][:])
        nc.sync.dma_start(out=T_PLIDX[:], in_=ins["plidx"][:])
        nc.sync.dma_start(out=T_RECIP[:], in_=ins["recip"][:])
        nc.sync.dma_start(out=T_W1R1[:], in_=ins["w1r1"][:])
        nc.sync.dma_start(out=T_WREP[:], in_=ins["wrep"][:])

        rsins = [dram.tile([128, L], f32, name=f"rsin{i}") for i in range(NLAYERS)]
        rsouts = [dram.tile([16, L], f32, name=f"rsout{i}") for i in range(NLAYERS)]
        arin = dram.tile([16, NLAYERS * G], f32)
        arout = dram.tile([16, NLAYERS * G], f32, addr_space="Shared")
        groups = [list(range(NCORES))]

        for li in range(1, NLAYERS + 1):
            # --- build replicated q^T table: q = h @ w1_l ---
            for co in range(0, L, MMCH):
                cw = min(MMCH, L - co)
                ps = pspool.tile([128, MMCH], f32)
                if li == 1:
                    xt = spool.tile([128, MMCH], f32, name="st")
                    nc.sync.dma_start(out=xt[:, :cw], in_=ins["xt"][:, co : co + cw])
                    nc.tensor.matmul(ps[:, :cw], lhsT=T_W1R1[:], rhs=xt[:, :cw],
                                     start=True, stop=True)
                else:
                    lw = T_WREP[64 : 64 + DIM, 128 * (li - 2) : 128 * (li - 1)]
                    nc.tensor.matmul(ps[:, :cw], lhsT=lw,
                                     rhs=T_SCR[64 : 64 + DIM, co : co + cw],
                                     start=True, stop=True)
                nc.scalar.copy(T_TAB[:, co : co + cw], ps[:, :cw])
            if debug and li == 1:
                nc.sync.dma_start(out=o_tab[:], in_=T_TAB[:, 0:L])

            # --- gather q[src] for all 8 dst-core groups; reduce per dst ---
            for (eoff, nd, c, poff) in chunks:
                nidx = nd * c
                if c == 1:
                    nc.gpsimd.ap_gather(
                        T_PART[:, poff : poff + nd], T_TAB[:],
                        T_EIDX[:, eoff // 16 : (eoff + nidx) // 16],
                        channels=128, num_elems=L + 16, d=1, num_idxs=nidx)
                else:
                    gt = wpool.tile([128, GW], f32, name="wt")
                    nc.gpsimd.ap_gather(
                        gt[:, :nidx], T_TAB[:],
                        T_EIDX[:, eoff // 16 : (eoff + nidx) // 16],
                        channels=128, num_elems=L + 16, d=1, num_idxs=nidx)
                    nc.vector.tensor_reduce(
                        T_PART[:, poff : poff + nd],
                        gt[:, :nidx].rearrange("p (n d) -> p n d", d=c),
                        axis=mybir.AxisListType.X, op=mybir.AluOpType.add)

            if debug and li == 1:
                nc.sync.dma_start(out=o_part[:], in_=T_PART[:, 0:NCLS])

            # --- permute class slots to dst-canonical cols; ReduceScatter ---
            for po in range(0, L, PCH):
                pw = min(PCH, L - po)
                pt = wpool.tile([128, max(GW, PCH)], f32, name="wt")
                nc.gpsimd.ap_gather(
                    pt[:, :pw], T_PART[:],
                    T_PIDX[:, po // 16 : (po + pw) // 16],
                    channels=128, num_elems=NCLS + 16, d=1, num_idxs=pw)
                nc.sync.dma_start(out=rsins[li - 1][:, po : po + pw], in_=pt[:, :pw])
            nc.gpsimd.collective_compute(
                "ReduceScatter", mybir.AluOpType.add, replica_groups=groups,
                ins=[rsins[li - 1][:].opt()], outs=[rsouts[li - 1][:].opt()])

            # --- combine: z = q + agg; u = relu(z+b1); h' = relu(w2^T u + b2) ---
            nc.sync.dma_start(out=T_SCR[0:DIM, :], in_=rsouts[li - 1][0:DIM, :])
            if debug and li == 1:
                nc.sync.dma_start(out=o_rs[:], in_=T_SCR[0:DIM, :])
            nc.vector.tensor_add(T_SCR[32 : 32 + DIM, :], T_TAB[0:DIM, 0:L],
                                 T_SCR[0:DIM, :])
            nc.scalar.activation(T_SCR[32 : 32 + DIM, :], T_SCR[32 : 32 + DIM, :],
                                 mybir.ActivationFunctionType.Relu,
                                 bias=T_WREP[32 : 32 + DIM, 50 + li - 1 : 50 + li])
            for co in range(0, L, MMCH):
                cw = min(MMCH, L - co)
                p2 = ps2pool.tile([16, MMCH], f32)
                nc.tensor.matmul(p2[0:DIM, :cw],
                                 lhsT=T_WREP[32 : 32 + DIM, DIM * (li - 1) : DIM * li],
                                 rhs=T_SCR[32 : 32 + DIM, co : co + cw],
                                 start=True, stop=True)
                nc.scalar.activation(T_SCR[64 : 64 + DIM, co : co + cw], p2[0:DIM, :cw],
                                     mybir.ActivationFunctionType.Relu,
                                     bias=T_WREP[0:DIM, 80 + li - 1 : 80 + li])

            if debug and li == 1:
                nc.sync.dma_start(out=o_h[:], in_=T_SCR[64 : 64 + DIM, 0:L])

            # --- per-graph pooling sums -> global gid positions ---
            for (goff, m_s, s, coloff) in size_classes:
                nc.vector.tensor_reduce(
                    T_HPOOL[0:DIM, goff : goff + m_s],
                    T_SCR[64 : 64 + DIM, coloff : coloff + m_s * s]
                    .rearrange("p (n d) -> p n d", d=s),
                    axis=mybir.AxisListType.X, op=mybir.AluOpType.add)
            pgt = spool.tile([128, MMCH], f32, name="st")
            pg = pgt[0:16, 0:G]
            nc.gpsimd.ap_gather(pg, T_HPOOL[:], T_PLIDX[:],
                                channels=16, num_elems=NGL + 16, d=1, num_idxs=G)
            nc.sync.dma_start(out=arin[:, G * (li - 1) : G * li], in_=pg)

        # --- AllReduce pooled sums; mean; heads; tanh ---
        nc.gpsimd.collective_compute(
            "AllReduce", mybir.AluOpType.add, replica_groups=groups,
            ins=[arin[:].opt()], outs=[arout[:].opt()])
        nc.sync.dma_start(out=T_SCR[0:16, 0 : NLAYERS * G], in_=arout[:])
        for li in range(1, NLAYERS + 1):
            nc.vector.tensor_mul(T_SCR[0:16, G * (li - 1) : G * li],
                                 T_SCR[0:16, G * (li - 1) : G * li], T_RECIP[:])
        psf = ps2pool.tile([16, G], f32)
        for li in range(1, NLAYERS + 1):
            nc.tensor.matmul(psf[:], lhsT=T_WREP[0:DIM, OUT * (li - 1) : OUT * li],
                             rhs=T_SCR[0:DIM, G * (li - 1) : G * li],
                             start=(li == 1), stop=(li == NLAYERS))
        fot = spool.tile([128, MMCH], f32, name="st")
        fo = fot[0:16, 0:G]
        nc.scalar.activation(fo, psf[:], mybir.ActivationFunctionType.Tanh)
        nc.sync.dma_start(out=o_out[:], in_=fo)

    return kern


def gin_run(inputs, G=G_TOTAL, sim=False, expected=None, tol=None, debug=False):
    """Build plan, run on 8 cores (sim or HW). Returns (out [G,OUT] f32|None, res)."""
    x = np.asarray(inputs["x"], np.float32)
    edge_index = np.asarray(inputs["edge_index"]).astype(np.int64)
    batch = np.asarray(inputs["batch"]).astype(np.int64)
    weights = {k: np.asarray(v, np.float32) for k, v in inputs.items()
               if k not in ("x", "edge_index", "batch")}
    plan = build_plan(x, edge_index, batch, weights, G)
    kern = make_kern(plan, debug=debug)
    exp_outs = None
    output_like = None
    if expected is not None:
        exp_outs = [[np.ascontiguousarray(expected.T.astype(np.float32))]
                    for _ in range(NCORES)]
    else:
        L, NCLS = plan["L"], plan["NCLS"]
        extra = ([np.zeros((128, L), np.float32), np.zeros((128, NCLS), np.float32),
                  np.zeros((DIM, L), np.float32), np.zeros((DIM, L), np.float32)]
                 if debug else [])
        output_like = [[np.zeros((OUT, G), np.float32)] + extra
                       for _ in range(NCORES)]
    kwargs = {} if tol is None else tol
    res = run_kernel(
        kern, exp_outs, plan["ins_list"], output_like=output_like,
        bass_type=tile.TileContext, num_cores=NCORES,
        check_with_sim=sim, check_with_hw=not sim,
        trace_sim=False, **kwargs)
    out = None
    if res is not None and res.results:
        out_t = next(iter(res.results[0].values()))
        out = np.ascontiguousarray(out_t.T.astype(np.float32))
    return out, res


def kernel(**inputs):
    global LAST_RESULTS
    sim = os.environ.get("GIN_SIM", "") == "1"
    out, res = gin_run(inputs, G=G_TOTAL, sim=sim)
    LAST_RESULTS = res
    return out
